# revision 12
# baseline (speedup 1.0000x reference)
"""Trainium2 Bass/Tile kernel for nn_CNN_77077483094746 (v4).

Single tiny sample (x: [1,1,18,140]) -> (1,2); whole forward on one core,
SPMD on 8 cores with identical inputs, output from core 0.

Measured facts driving the design (NTFF traces on this part):
- HBM->SBUF DMA pays ~2us PER DESCRIPTOR per engine (not pipelined within an
  engine); SBUF->SBUF descriptors cost ~0.45us/engine. So: ONE bf16 HBM load
  shaped [16, M] (16 fat descriptors), then two row-half SBUF reshapes (64
  descriptors each) running on both HWDGE queues in parallel.
- qp/kp biases are folded into the projection matmuls as a K=119 contraction
  (bias row appended to the weights, ones row appended to eegT/kAT/kBT) so
  no f32 bias columns are needed early.
- DVE ops on single-partition rows serialize per lane (a [1,118] reciprocal
  costs ~880ns). The softmax normalizer is computed as [118,4] columns (one
  ~60ns reciprocal), PE-transposed to [4,118], spread to a [64,118] mask by
  one indicator matmul.
- All weight reformatting on HOST (pre-transposed/scaled/folded, bf16).
- wA/wB are rank-1, so every stage-2 score matrix is rank-1, materialized
  TRANSPOSED by one K=1 outer-product matmul; no [118,118] transposes.
- Wo + output-side biases folded into conv weights / relu bias on host.
- Stage-1 select: s = ncol^T M2 + csel with M2/Zw/GT attention-independent.
- attn max-subtraction dropped (|S| < 2 measured); sigmoid via exp table.
"""
import math
from contextlib import ExitStack

import numpy as np
import ml_dtypes

import concourse.bass as bass
import concourse.mybir as mybir
import concourse.tile as tile
from concourse import bacc
from concourse.bass_utils import run_bass_kernel_spmd
from concourse.masks import make_identity

WL = 140
OFC = 118
TDN = 21
D_CM = 16
N_BR = 4
C_OUT = 10
KS = 9
NCONV = OFC - KS + 1
F32 = mybir.dt.float32
BF16 = mybir.dt.bfloat16
N_CORES = 8
BF = ml_dtypes.bfloat16

INPUT_SPECS = {
    "x": (1, 1, 18, WL),
    "tdA_in_w": (3 * OFC, OFC), "tdA_in_b": (3 * OFC,),
    "tdA_out_w": (OFC, OFC), "tdA_out_b": (OFC,),
    "tdB_in_w": (3 * OFC, OFC), "tdB_in_b": (3 * OFC,),
    "tdB_out_w": (OFC, OFC), "tdB_out_b": (OFC,),
    "cm_in_w": (N_BR, 3 * D_CM, D_CM), "cm_in_b": (N_BR, 3 * D_CM),
    "cm_out_w": (N_BR, D_CM, D_CM), "cm_out_b": (N_BR, D_CM),
    "projA_w": (16, 1), "projB_w": (16, 1),
    "conv_w": (N_BR, C_OUT, 16, KS), "conv_b": (N_BR, C_OUT),
    "fc1_w": (40, 40), "fc1_b": (40,),
    "fc2_w": (2, 40), "fc2_b": (2,),
}

# ---------------- wb bf16 [128, 1960] column layout -------------------------
C_EEGT = 0        # [0:119, 0:16]   eegT; row 118 = 1.0 (bias fold)
C_KAT = 16        # [0:119, 16:37]  kA^T; row 118 = 1.0
C_KBT = 37        # [0:119, 37:58]  kB^T; row 118 = 1.0
C_QKV_A = 58      # [0:119, 58:412] WqT|WkT|WvT; row 118 = bq|bk|0
C_QKV_B = 412
C_OB16_A = 766    # [118,1] 16*obr
C_OB16_B = 767
C_WO_A = 768      # [118,118] Wo as stored [o,c]
C_WOT_A = 886     # [118,118] Wo^T [c,o]
C_WO_B = 1004
C_WOT_B = 1122
C_ONES118 = 1240  # [118,1] ones
C_CONV = 1241     # rows 0:64, 9 slices of 40
C_FC1 = 1601      # rows 0:40 [40,40]
C_FC2 = 1641      # rows 0:40 [40,2]
C_EEG = 1643      # rows 0:16 [16,118] eeg natural
C_WQ1T = 1761     # rows 0:16 [16,16]
C_WQ2T = 1777
C_WK0T = 1793
C_WK3T = 1809
C_WV03 = 1825     # rows 0:16 [16,32]
C_UQ0, C_BQ0, C_UQ3, C_BQ3, C_UK1, C_UK2 = 1857, 1858, 1859, 1860, 1861, 1862
C_VV1 = 1863      # row 0 [1,16]
C_VV2 = 1879
C_EE = 1895       # rows 0:4 [4,64]
NWB = 1960
# pf f32 [48, 242]
PF_NEGB1, PF_NEGB2, PF_CCONST = 0, 1, 2
PF_KAP1, PF_KAP2, PF_ID1 = 3, 4, 5    # row-0 scalars
PF_OBR_A = (6, 124)                   # row 0 [1,118]
PF_OBR_B = (124, 242)
NPF = 242


def host_pack(I):
    """Weight-only reformatting + x marshaling -> ([16,*] bf16, [16,*] f32)."""
    f32 = np.float32
    wb = np.zeros((128, NWB), f32)
    pf = np.zeros((48, NPF), f32)
    x = np.asarray(I["x"], f32)[0, 0]
    idx = np.arange(TDN)[:, None] + np.arange(OFC)[None, :]
    eeg = x[1:17, WL - OFC:]
    wb[0:OFC, C_EEGT:C_EEGT + 16] = eeg.T
    wb[0:OFC, C_KAT:C_KAT + TDN] = x[0][idx].T
    wb[0:OFC, C_KBT:C_KBT + TDN] = x[17][idx].T
    wb[OFC, C_EEGT:C_EEGT + 58] = 1.0      # ones row for bias folds
    wb[0:16, C_EEG:C_EEG + OFC] = eeg

    s1 = 1.0 / math.sqrt(OFC)
    for bi, p in enumerate(("tdA", "tdB")):
        inw = np.asarray(I[f"{p}_in_w"], f32)
        inb = np.asarray(I[f"{p}_in_b"], f32)
        outw = np.asarray(I[f"{p}_out_w"], f32)
        outb = np.asarray(I[f"{p}_out_b"], f32)
        o = C_QKV_A if bi == 0 else C_QKV_B
        wb[0:OFC, o:o + 118] = inw[0:OFC].T * s1
        wb[0:OFC, o + 118:o + 236] = inw[OFC:2 * OFC].T
        wb[0:OFC, o + 236:o + 354] = inw[2 * OFC:3 * OFC].T
        wb[OFC, o:o + 118] = inb[0:OFC] * s1           # bq row
        wb[OFC, o + 118:o + 236] = inb[OFC:2 * OFC]    # bk row
        bv = inb[2 * OFC:3 * OFC]
        obr = bv @ outw.T + outb
        r = PF_OBR_A if bi == 0 else PF_OBR_B
        pf[0, r[0]:r[1]] = obr
        wb[0:OFC, (C_OB16_A if bi == 0 else C_OB16_B)] = 16.0 * obr
        wo = C_WO_A if bi == 0 else C_WO_B
        wot = C_WOT_A if bi == 0 else C_WOT_B
        wb[0:OFC, wo:wo + 118] = outw
        wb[0:OFC, wot:wot + 118] = outw.T
    wb[0:OFC, C_ONES118] = 1.0
    for i in range(4):
        wb[i, C_EE + 16 * i:C_EE + 16 * (i + 1)] = 1.0
    pf[0, PF_ID1] = 1.0

    cmw = np.asarray(I["cm_in_w"], f32)
    cmb = np.asarray(I["cm_in_b"], f32)
    cow = np.asarray(I["cm_out_w"], f32)
    cob = np.asarray(I["cm_out_b"], f32)
    pA = np.asarray(I["projA_w"], f32)[:, 0]
    pB = np.asarray(I["projB_w"], f32)[:, 0]
    s2 = 1.0 / math.sqrt(D_CM)
    wq, wk, wv = cmw[:, 0:16], cmw[:, 16:32], cmw[:, 32:48]
    bq, bv2 = cmb[:, 0:16], cmb[:, 32:48]
    wb[0:16, C_WQ1T:C_WQ1T + 16] = wq[1].T * s2
    wb[0:16, C_WQ2T:C_WQ2T + 16] = wq[2].T * s2
    wb[0:16, C_WK0T:C_WK0T + 16] = wk[0].T
    wb[0:16, C_WK3T:C_WK3T + 16] = wk[3].T
    wb[0:16, C_WV03:C_WV03 + 16] = wv[0].T
    wb[0:16, C_WV03 + 16:C_WV03 + 32] = wv[3].T
    wb[0:16, C_UQ0] = (wq[0] @ pA) * s2
    wb[0:16, C_BQ0] = bq[0] * s2
    wb[0:16, C_UQ3] = (wq[3] @ pB) * s2
    wb[0:16, C_BQ3] = bq[3] * s2
    wb[0:16, C_UK1] = wk[1] @ pA
    wb[0:16, C_UK2] = wk[2] @ pB
    wb[0, C_VV1:C_VV1 + 16] = wv[1] @ pA
    wb[0, C_VV2:C_VV2 + 16] = wv[2] @ pB
    pf[0, PF_KAP1] = (bq[1] * s2) @ (wk[1] @ pA)
    pf[0, PF_KAP2] = (bq[2] * s2) @ (wk[2] @ pB)

    convw = np.asarray(I["conv_w"], f32)
    obeff = np.stack([cow[i] @ bv2[i] + cob[i] for i in range(4)])
    const = np.asarray(I["conv_b"], f32).reshape(-1).copy()
    for i in range(4):
        for k in range(KS):
            wb[16 * i:16 * (i + 1),
               C_CONV + 40 * k + 10 * i:C_CONV + 40 * k + 10 * (i + 1)] = (
                convw[i, :, :, k] @ cow[i]).T
        const[10 * i:10 * (i + 1)] += np.einsum("ock,c->o", convw[i], obeff[i])
    pf[0:40, PF_CCONST] = const
    wb[0:40, C_FC1:C_FC1 + 40] = np.asarray(I["fc1_w"], f32).T
    wb[0:40, C_FC2:C_FC2 + 2] = np.asarray(I["fc2_w"], f32).T
    pf[0:40, PF_NEGB1] = -np.asarray(I["fc1_b"], f32)
    pf[0:2, PF_NEGB2] = -np.asarray(I["fc2_b"], f32)

    return wb.astype(BF).reshape(16, -1), pf.reshape(16, -1)


def _emit(nc, tc, H, out_ap):
    AF = mybir.ActivationFunctionType
    ALU = mybir.AluOpType
    X = mybir.AxisListType.X

    ctx = ExitStack()
    consts = ctx.enter_context(tc.tile_pool(name="consts", bufs=1))
    work = ctx.enter_context(tc.tile_pool(name="work", bufs=1))
    psum = ctx.enter_context(tc.tile_pool(name="psum", bufs=1, space="PSUM"))

    def dram_ap(handle, dims):
        return bass.AP(tensor=handle, offset=0, ap=[list(d) for d in dims])

    def pst(shape, nm, tag, bufs=2):
        return psum.tile(shape, F32, name=nm, tag=tag, bufs=bufs)

    # --------- HBM loads ([16,M], 16 fat descriptors) + row-half reshapes ---
    st_wb = consts.tile([16, NWB * 8], BF16, name="st_wb")
    nc.sync.dma_start(out=st_wb[:, :],
                      in_=dram_ap(H["wbx"], [(NWB * 8, 16), (1, NWB * 8)]))
    st_pf = consts.tile([16, NPF * 3], F32, name="st_pf")
    nc.scalar.dma_start(out=st_pf[:, :],
                        in_=dram_ap(H["pfx"], [(NPF * 3, 16), (1, NPF * 3)]))
    wb = consts.tile([128, NWB], BF16, name="wb")
    nc.sync.dma_start(out=wb[0:64, :], in_=st_wb[0:8, :])
    nc.scalar.dma_start(out=wb[64:128, :], in_=st_wb[8:16, :])
    pf = consts.tile([48, NPF], F32, name="pf")
    nc.scalar.dma_start(out=pf[:, :], in_=st_pf[:, :])

    id118 = consts.tile([OFC, OFC], F32, name="id118")
    make_identity(nc, id118)
    vpads = work.tile([OFC, 256], BF16, name="vpads")

    eegT = wb[0:OFC, C_EEGT:C_EEGT + 16]
    eegT1 = wb[0:OFC + 1, C_EEGT:C_EEGT + 16]       # with ones row
    kT = {"A": wb[0:OFC, C_KAT:C_KAT + TDN], "B": wb[0:OFC, C_KBT:C_KBT + TDN]}
    kT1 = {"A": wb[0:OFC + 1, C_KAT:C_KAT + TDN],
           "B": wb[0:OFC + 1, C_KBT:C_KBT + TDN]}
    eeg = wb[0:16, C_EEG:C_EEG + OFC]

    s1 = {"A": {}, "B": {}}
    tagm = {"A": "tA", "B": "tB"}
    qkvo = {"A": C_QKV_A, "B": C_QKV_B}
    woN = {"A": wb[0:OFC, C_WO_A:C_WO_A + 118], "B": wb[0:OFC, C_WO_B:C_WO_B + 118]}
    woT = {"A": wb[0:OFC, C_WOT_A:C_WOT_A + 118],
           "B": wb[0:OFC, C_WOT_B:C_WOT_B + 118]}
    ob16 = {"A": wb[0:OFC, C_OB16_A:C_OB16_A + 1],
            "B": wb[0:OFC, C_OB16_B:C_OB16_B + 1]}
    obrr = {"A": pf[0:1, PF_OBR_A[0]:PF_OBR_A[1]],
            "B": pf[0:1, PF_OBR_B[0]:PF_OBR_B[1]]}
    kapc = {1: pf[0:1, PF_KAP1:PF_KAP1 + 1], 2: pf[0:1, PF_KAP2:PF_KAP2 + 1]}
    id1 = pf[0:1, PF_ID1:PF_ID1 + 1]
    s_ps = pst([1, 32], "s_ps", "tS", bufs=1)  # A cols 0:16, B cols 16:32

    def ps1(br, shape, nm):
        return pst(shape, f"{nm}_{br}", tagm[br])

    def qk_mms(br):
        d = s1[br]
        o = qkvo[br]
        d["qpT_ps"] = ps1(br, [OFC, 16], "qpT")
        nc.tensor.matmul(d["qpT_ps"][:, :], wb[0:OFC + 1, o:o + 118], eegT1)
        d["kpT_ps"] = ps1(br, [OFC, TDN], "kpT")
        nc.tensor.matmul(d["kpT_ps"][:, :], wb[0:OFC + 1, o + 118:o + 236], kT1[br])

    def qk_cps(br):
        d = s1[br]
        d["qpT"] = work.tile([OFC, 16], BF16, name=f"qpT_{br}")
        nc.vector.tensor_copy(d["qpT"][:, :], d["qpT_ps"][:, :])
        d["kpT"] = work.tile([OFC, TDN], BF16, name=f"kpT_{br}")
        nc.vector.tensor_copy(d["kpT"][:, :], d["kpT_ps"][:, :])

    def vphT_mm(br):
        d = s1[br]
        o = qkvo[br]
        d["vphT_ps"] = ps1(br, [OFC, TDN], "vphT")
        nc.tensor.matmul(d["vphT_ps"][:, :], wb[0:OFC, o + 236:o + 354], kT[br])

    def vphT_cp(br):
        d = s1[br]
        d["vphT"] = work.tile([OFC, TDN], BF16, name=f"vphT_{br}")
        nc.scalar.copy(d["vphT"][:, :], d["vphT_ps"][:, :])

    def s_mm(br):
        d = s1[br]
        d["S_ps"] = ps1(br, [16, TDN], "S")
        nc.tensor.matmul(d["S_ps"][:, :], d["qpT"][:, :], d["kpT"][:, :])

    def csel_mm(br):
        off = 0 if br == "A" else 16
        nc.tensor.matmul(s_ps[0:1, off:off + 16], ob16[br], eegT,
                         start=True, stop=False)

    def gt_mm(br):
        d = s1[br]
        d["GT_ps"] = ps1(br, [OFC, 16], "GT")
        nc.tensor.matmul(d["GT_ps"][:, :], woN[br], eegT)

    def gt_cp(br):
        d = s1[br]
        d["GT"] = work.tile([OFC, 16], BF16, name=f"GT_{br}")
        nc.scalar.copy(d["GT"][:, :], d["GT_ps"][:, :])

    def softmax1(br):
        d = s1[br]
        d["P"] = work.tile([16, TDN], F32, name=f"P_{br}")
        d["rowsum"] = work.tile([16, 1], F32, name=f"rowsum_{br}")
        nc.scalar.activation(d["P"][:, :], d["S_ps"][:, :], AF.Exp,
                             scale=1.0, accum_out=d["rowsum"][:, :])
        d["rinv"] = work.tile([16, 1], F32, name=f"rinv_{br}")
        nc.vector.reciprocal(d["rinv"][:, :], d["rowsum"][:, :])

    def ncol_mm(br):
        d = s1[br]
        d["ncol_ps"] = ps1(br, [TDN, 1], "ncol")
        nc.tensor.matmul(d["ncol_ps"][:, :], d["P"][:, :], d["rinv"][:, :])

    def ncol_cp(br):
        d = s1[br]
        d["ncol"] = work.tile([TDN, 1], BF16, name=f"ncol_{br}")
        nc.vector.tensor_copy(d["ncol"][:, :], d["ncol_ps"][:, :])

    def m2_mm(br):
        d = s1[br]
        d["M2_ps"] = ps1(br, [TDN, 16], "M2")
        nc.tensor.matmul(d["M2_ps"][:, :], d["vphT"][:, :], d["GT"][:, :])

    def m2_cp(br):
        d = s1[br]
        d["M2"] = work.tile([TDN, 16], BF16, name=f"M2_{br}")
        nc.scalar.copy(d["M2"][:, :], d["M2_ps"][:, :])

    def zw_mm(br):
        d = s1[br]
        d["Zw_ps"] = ps1(br, [TDN, OFC], "Zw")
        nc.tensor.matmul(d["Zw_ps"][:, :], d["vphT"][:, :], woT[br])

    def zw_cp(br):
        d = s1[br]
        d["Zw"] = work.tile([TDN, OFC], BF16, name=f"Zw_{br}")
        nc.scalar.copy(d["Zw"][:, :], d["Zw_ps"][:, :])

    def s_mm2(br):
        d = s1[br]
        off = 0 if br == "A" else 16
        nc.tensor.matmul(s_ps[0:1, off:off + 16], d["ncol"][:, :], d["M2"][:, :],
                         start=False, stop=True)

    def sel_post(br):
        d = s1[br]
        off = 0 if br == "A" else 16
        d["m"] = work.tile([1, 1], F32, name=f"m_{br}")
        nc.vector.reduce_max(d["m"][:, :], s_ps[0:1, off:off + 16], axis=X)
        d["oh"] = work.tile([1, 16], F32, name=f"oh_{br}")
        nc.vector.tensor_scalar(d["oh"][:, :], s_ps[0:1, off:off + 16],
                                d["m"][:, :], None, op0=ALU.is_equal)

    def oht_mm(br):
        d = s1[br]
        d["ohT_ps"] = ps1(br, [16, 1], "ohT")
        nc.tensor.transpose(d["ohT_ps"][:, :], d["oh"][:, :], id1)

    def rh_cp(br):
        d = s1[br]
        d["rh"] = work.tile([16, 1], F32, name=f"rh_{br}")
        nc.vector.tensor_mul(d["rh"][:, :], d["ohT_ps"][:, :], d["rinv"][:, :])

    def nsel_mm(br):
        d = s1[br]
        d["nsel_ps"] = ps1(br, [TDN, 1], "nsel")
        nc.tensor.matmul(d["nsel_ps"][:, :], d["P"][:, :], d["rh"][:, :])

    def nsel_cp(br):
        d = s1[br]
        d["nsel"] = work.tile([TDN, 1], BF16, name=f"nsel_{br}")
        nc.vector.tensor_copy(d["nsel"][:, :], d["nsel_ps"][:, :])

    def row_mm(br):
        d = s1[br]
        d["row_ps"] = ps1(br, [1, OFC], "row")
        nc.tensor.matmul(d["row_ps"][:, :], d["nsel"][:, :], d["Zw"][:, :])

    def row_cp(br):
        d = s1[br]
        d["row"] = work.tile([1, OFC], BF16, name=f"row_{br}")
        nc.vector.tensor_add(d["row"][:, :], d["row_ps"][:, :], obrr[br])

    e = {}

    def eproj(nm, col):
        ps = pst([16, OFC], f"{nm}_ps", "tE")
        nc.tensor.matmul(ps[:, :], wb[0:16, col:col + 16], eeg)
        e[nm + "_ps"] = ps

    def eproj_cp(nm):
        t = work.tile([16, OFC], BF16, name=nm)
        nc.scalar.copy(t[:, :], e[nm + "_ps"][:, :])
        e[nm] = t

    def factor_mm(nm, lhsT, rhs, shape):
        ps = pst(shape, f"{nm}_ps", "tE")
        nc.tensor.matmul(ps[:, :], lhsT, rhs)
        e[nm + "_ps"] = ps

    # ---------------- emission order ----------------------------------------
    qk_mms("A")
    qk_mms("B")
    qk_cps("A")
    nc.vector.memset(vpads[:, :], 0.0)
    vphT_mm("A")
    vphT_mm("B")
    qk_cps("B")
    s_mm("A")
    csel_mm("A")
    vphT_cp("A")
    s_mm("B")
    csel_mm("B")
    vphT_cp("B")
    softmax1("A")
    gt_mm("A")
    gt_mm("B")
    softmax1("B")
    gt_cp("A")
    gt_cp("B")
    ncol_mm("A")
    m2_mm("A")
    ncol_cp("A")
    ncol_mm("B")
    m2_mm("B")
    ncol_cp("B")
    m2_cp("A")
    m2_cp("B")
    eproj("qp1T", C_WQ1T)
    zw_mm("A")
    s_mm2("A")
    eproj_cp("qp1T")
    zw_mm("B")
    sel_post("A")
    zw_cp("A")
    eproj("kp0T", C_WK0T)
    s_mm2("B")
    oht_mm("A")
    sel_post("B")
    rh_cp("A")
    zw_cp("B")
    eproj_cp("kp0T")
    nsel_mm("A")
    oht_mm("B")
    nsel_cp("A")
    rh_cp("B")
    eproj("qp2T", C_WQ2T)
    nsel_mm("B")
    eproj_cp("qp2T")
    row_mm("A")
    nsel_cp("B")
    row_cp("A")
    eproj("kp3T", C_WK3T)
    row_mm("B")
    eproj_cp("kp3T")
    row_cp("B")
    rowS = {"A": s1["A"]["row"], "B": s1["B"]["row"]}
    # rank-1 factors + vp0/vp3 (eeg-side): independent of rowA/rowB, emitted
    # before the row-dependent matmuls so they fill select-phase gaps
    vp03_ps = pst([OFC, 32], "vp03_ps", "tE")
    nc.tensor.matmul(vp03_ps[:, :], eeg, wb[0:16, C_WV03:C_WV03 + 32])
    nc.vector.tensor_copy(vpads[:, 0:16], vp03_ps[:, 0:16])
    nc.vector.tensor_copy(vpads[:, 240:256], vp03_ps[:, 16:32])
    factor_mm("g0", wb[0:16, C_UQ0:C_UQ0 + 1], e["kp0T"][:, :], [1, OFC])
    g0 = work.tile([1, OFC], BF16, name="g0")
    nc.vector.tensor_copy(g0[:, :], e["g0_ps"][:, :])
    factor_mm("c0", e["kp0T"][:, :], wb[0:16, C_BQ0:C_BQ0 + 1], [OFC, 1])
    c0 = work.tile([OFC, 1], F32, name="c0")
    nc.scalar.copy(c0[:, :], e["c0_ps"][:, :])
    factor_mm("h1", wb[0:16, C_UK1:C_UK1 + 1], e["qp1T"][:, :], [1, OFC])
    h1 = work.tile([1, OFC], BF16, name="h1")
    nc.vector.tensor_scalar_add(h1[:, :], e["h1_ps"][:, :], kapc[1])
    factor_mm("g3", wb[0:16, C_UQ3:C_UQ3 + 1], e["kp3T"][:, :], [1, OFC])
    g3 = work.tile([1, OFC], BF16, name="g3")
    nc.vector.tensor_copy(g3[:, :], e["g3_ps"][:, :])
    factor_mm("c3", e["kp3T"][:, :], wb[0:16, C_BQ3:C_BQ3 + 1], [OFC, 1])
    c3 = work.tile([OFC, 1], F32, name="c3")
    nc.scalar.copy(c3[:, :], e["c3_ps"][:, :])
    factor_mm("h2", wb[0:16, C_UK2:C_UK2 + 1], e["qp2T"][:, :], [1, OFC])
    h2 = work.tile([1, OFC], BF16, name="h2")
    nc.vector.tensor_scalar_add(h2[:, :], e["h2_ps"][:, :], kapc[2])

    # rowA/rowB dependent
    vp1_ps = pst([OFC, 16], "vp1_ps", "tS", bufs=1)
    nc.tensor.matmul(vp1_ps[:, :], rowS["A"][:, :], wb[0:1, C_VV1:C_VV1 + 16])
    nc.vector.tensor_copy(vpads[:, 80:96], vp1_ps[:, :])
    pt_ps = [None] * 4
    pt_ps[0] = pst([OFC, OFC], "pt0_ps", "tA")
    nc.tensor.matmul(pt_ps[0][:, :], g0[:, :], rowS["A"][:, :])
    pt_ps[1] = pst([OFC, OFC], "pt1_ps", "tA")
    nc.tensor.matmul(pt_ps[1][:, :], rowS["A"][:, :], h1[:, :])
    vp2_ps = pst([OFC, 16], "vp2_ps", "tS", bufs=1)
    nc.tensor.matmul(vp2_ps[:, :], rowS["B"][:, :], wb[0:1, C_VV2:C_VV2 + 16])
    nc.vector.tensor_copy(vpads[:, 160:176], vp2_ps[:, :])
    pt_ps[2] = pst([OFC, OFC], "pt2_ps", "tB")
    nc.tensor.matmul(pt_ps[2][:, :], rowS["B"][:, :], h2[:, :])
    pt_ps[3] = pst([OFC, OFC], "pt3_ps", "tB")
    nc.tensor.matmul(pt_ps[3][:, :], g3[:, :], rowS["B"][:, :])

    ptall = work.tile([OFC, 4 * OFC], BF16, name="ptall")
    biases = [c0, None, None, c3]
    for i in range(4):
        b = biases[i]
        nc.scalar.activation(ptall[:, OFC * i:OFC * (i + 1)], pt_ps[i][:, :],
                             AF.Exp, bias=(b[:, :] if b is not None else 0.0),
                             scale=1.0)

    ztall_ps = pst([64, OFC], "ztall_ps", "tZ", bufs=1)
    rs_ps = pst([OFC, 4], "rs_ps", "tS", bufs=1)
    ones118 = wb[0:OFC, C_ONES118:C_ONES118 + 1]
    for i in range(4):
        nc.tensor.matmul(rs_ps[:, i:i + 1],
                         ptall[:, OFC * i:OFC * (i + 1)], ones118)
        nc.tensor.matmul(ztall_ps[:, :], vpads[:, 64 * i:64 * (i + 1)],
                         ptall[:, OFC * i:OFC * (i + 1)],
                         start=(i == 0), stop=(i == 3))
    rinv_col = work.tile([OFC, 4], F32, name="rinv_col")
    nc.vector.reciprocal(rinv_col[:, :], rs_ps[:, :])
    r4t_ps = pst([4, OFC], "r4t_ps", "tE")
    nc.tensor.transpose(r4t_ps[:, :], rinv_col[:, :], id118[:, :])
    r4 = work.tile([4, OFC], BF16, name="r4")
    nc.vector.tensor_copy(r4[:, :], r4t_ps[:, :])
    m_ps = pst([64, OFC], "m_ps", "tE")
    nc.tensor.matmul(m_ps[:, :], wb[0:4, C_EE:C_EE + 64], r4[:, :])
    m_sb = work.tile([64, OFC], F32, name="m_sb")
    nc.vector.tensor_copy(m_sb[:, :], m_ps[:, :])
    ztn = work.tile([64, OFC], BF16, name="ztn")
    nc.vector.tensor_mul(ztn[:, :], ztall_ps[:, :], m_sb[:, :])

    # ---------------- conv + head ------------------------------------------
    y_ps = pst([4 * C_OUT, NCONV], "y_ps", "tZ", bufs=1)
    for k in range(KS):
        nc.tensor.matmul(y_ps[:, :],
                         wb[0:64, C_CONV + 40 * k:C_CONV + 40 * (k + 1)],
                         ztn[:, k:k + NCONV], start=(k == 0), stop=(k == KS - 1))
    relu = work.tile([4 * C_OUT, NCONV], F32, name="relu")
    nc.scalar.activation(relu[:, :], y_ps[:, :], AF.Relu,
                         bias=pf[0:40, PF_CCONST:PF_CCONST + 1], scale=1.0)
    feat = work.tile([4 * C_OUT, 1], BF16, name="feat")
    nc.vector.reduce_max(feat[:, :], relu[:, :], axis=X)

    h_ps = pst([40, 1], "h_ps", "tZ", bufs=1)
    nc.tensor.matmul(h_ps[:, :], wb[0:40, C_FC1:C_FC1 + 40], feat[:, :])
    eh = work.tile([40, 1], F32, name="eh")
    nc.scalar.activation(eh[:, :], h_ps[:, :], AF.Exp,
                         bias=pf[0:40, PF_NEGB1:PF_NEGB1 + 1], scale=-1.0)
    eh1 = work.tile([40, 1], F32, name="eh1")
    nc.vector.tensor_scalar(eh1[:, :], eh[:, :], 1.0, None, op0=ALU.add)
    hsb = work.tile([40, 1], BF16, name="hsb")
    with nc.allow_low_precision(reason="bf16 operand for the 2x40 head matmul"):
        nc.vector.reciprocal(hsb[:, :], eh1[:, :])
    o_ps = pst([2, 1], "o_ps", "tZ", bufs=1)
    nc.tensor.matmul(o_ps[:, :], wb[0:40, C_FC2:C_FC2 + 2], hsb[:, :])
    eo = work.tile([2, 1], F32, name="eo")
    nc.scalar.activation(eo[:, :], o_ps[:, :], AF.Exp,
                         bias=pf[0:2, PF_NEGB2:PF_NEGB2 + 1], scale=-1.0)
    eo1 = work.tile([2, 1], F32, name="eo1")
    nc.vector.tensor_scalar(eo1[:, :], eo[:, :], 1.0, None, op0=ALU.add)
    res = work.tile([2, 1], F32, name="res")
    nc.vector.reciprocal(res[:, :], eo1[:, :])
    nc.sync.dma_start(out=out_ap, in_=res[:, :])
    ctx.close()


_CACHE = {}


def build():
    if "nc" in _CACHE:
        return _CACHE["nc"]
    nc = bacc.Bacc("TRN2", target_bir_lowering=False, debug=False,
                   num_devices=N_CORES)
    H = {
        "wbx": nc.dram_tensor("wbx", [16, NWB * 8], BF16, kind="ExternalInput"),
        "pfx": nc.dram_tensor("pfx", [16, NPF * 3], F32, kind="ExternalInput"),
    }
    out_t = nc.dram_tensor("out", [1, 2], F32, kind="ExternalOutput")
    with tile.TileContext(nc) as tc:
        _emit(nc, tc, H, out_t.ap())
    nc.compile()
    _CACHE["nc"] = nc
    return nc


def pack_inputs(inputs):
    wbx, pfx = host_pack(inputs)
    return {"wbx": np.ascontiguousarray(wbx), "pfx": np.ascontiguousarray(pfx)}


def kernel(**inputs):
    in_map = pack_inputs(inputs)
    nc = build()
    res = run_bass_kernel_spmd(nc, [in_map] * N_CORES,
                               core_ids=list(range(N_CORES)))
    return res.results[0]["out"]


# revision 13
# speedup vs baseline: 1.4383x; 1.4383x over previous
"""Trainium2 Bass/Tile kernel for nn_CNN_77077483094746 (v4).

Single tiny sample (x: [1,1,18,140]) -> (1,2); whole forward on one core,
SPMD on 8 cores with identical inputs, output from core 0.

Measured facts driving the design (NTFF traces on this part):
- HBM->SBUF DMA pays ~2us PER DESCRIPTOR per engine (not pipelined within an
  engine); SBUF->SBUF descriptors cost ~0.45us/engine. So: ONE bf16 HBM load
  shaped [16, M] (16 fat descriptors), then two row-half SBUF reshapes (64
  descriptors each) running on both HWDGE queues in parallel.
- qp/kp biases are folded into the projection matmuls as a K=119 contraction
  (bias row appended to the weights, ones row appended to eegT/kAT/kBT) so
  no f32 bias columns are needed early.
- DVE ops on single-partition rows serialize per lane (a [1,118] reciprocal
  costs ~880ns). The softmax normalizer is computed as [118,4] columns (one
  ~60ns reciprocal), PE-transposed to [4,118], spread to a [64,118] mask by
  one indicator matmul.
- All weight reformatting on HOST (pre-transposed/scaled/folded, bf16).
- wA/wB are rank-1, so every stage-2 score matrix is rank-1, materialized
  TRANSPOSED by one K=1 outer-product matmul; no [118,118] transposes.
- Wo + output-side biases folded into conv weights / relu bias on host.
- Stage-1 select: s = ncol^T M2 + csel with M2/Zw/GT attention-independent.
- attn max-subtraction dropped (|S| < 2 measured); sigmoid via exp table.
"""
import math
from contextlib import ExitStack

import numpy as np
import ml_dtypes

import concourse.bass as bass
import concourse.mybir as mybir
import concourse.tile as tile
from concourse import bacc
from concourse.bass_utils import run_bass_kernel_spmd
from concourse.masks import make_identity

WL = 140
OFC = 118
TDN = 21
D_CM = 16
N_BR = 4
C_OUT = 10
KS = 9
NCONV = OFC - KS + 1
F32 = mybir.dt.float32
BF16 = mybir.dt.bfloat16
N_CORES = 8
BF = ml_dtypes.bfloat16

INPUT_SPECS = {
    "x": (1, 1, 18, WL),
    "tdA_in_w": (3 * OFC, OFC), "tdA_in_b": (3 * OFC,),
    "tdA_out_w": (OFC, OFC), "tdA_out_b": (OFC,),
    "tdB_in_w": (3 * OFC, OFC), "tdB_in_b": (3 * OFC,),
    "tdB_out_w": (OFC, OFC), "tdB_out_b": (OFC,),
    "cm_in_w": (N_BR, 3 * D_CM, D_CM), "cm_in_b": (N_BR, 3 * D_CM),
    "cm_out_w": (N_BR, D_CM, D_CM), "cm_out_b": (N_BR, D_CM),
    "projA_w": (16, 1), "projB_w": (16, 1),
    "conv_w": (N_BR, C_OUT, 16, KS), "conv_b": (N_BR, C_OUT),
    "fc1_w": (40, 40), "fc1_b": (40,),
    "fc2_w": (2, 40), "fc2_b": (2,),
}

# ---------------- wb bf16 [128, 1960] column layout -------------------------
C_EEGT = 0        # [0:119, 0:16]   eegT; row 118 = 1.0 (bias fold)
C_KAT = 16        # [0:119, 16:37]  kA^T; row 118 = 1.0
C_KBT = 37        # [0:119, 37:58]  kB^T; row 118 = 1.0
C_QKV_A = 58      # [0:119, 58:412] WqT|WkT|WvT; row 118 = bq|bk|0
C_QKV_B = 412
C_OB16_A = 766    # [118,1] 16*obr
C_OB16_B = 767
C_WO_A = 768      # [118,118] Wo as stored [o,c]
C_WOT_A = 886     # [118,118] Wo^T [c,o]
C_WO_B = 1004
C_WOT_B = 1122
C_ONES118 = 1240  # [118,1] ones
C_CONV = 1241     # rows 0:64, 9 slices of 40
C_FC1 = 1601      # rows 0:40 [40,40]
C_FC2 = 1641      # rows 0:40 [40,2]
C_EEG = 1643      # rows 0:16 [16,118] eeg natural
C_WQ1T = 1761     # rows 0:16 [16,16]
C_WQ2T = 1777
C_WK0T = 1793
C_WK3T = 1809
C_WV03 = 1825     # rows 0:16 [16,32]
C_UQ0, C_BQ0, C_UQ3, C_BQ3, C_UK1, C_UK2 = 1857, 1858, 1859, 1860, 1861, 1862
C_VV1 = 1863      # row 0 [1,16]
C_VV2 = 1879
C_EE = 1895       # rows 0:4 [4,64]
NWB = 1960
# pf f32 [48, 242]
PF_NEGB1, PF_NEGB2, PF_CCONST = 0, 1, 2
PF_KAP1, PF_KAP2, PF_ID1 = 3, 4, 5    # row-0 scalars
PF_OBR_A = (6, 124)                   # row 0 [1,118]
PF_OBR_B = (124, 242)
NPF = 242


def host_pack(I):
    """Weight-only reformatting + x marshaling -> ([16,*] bf16, [16,*] f32)."""
    f32 = np.float32
    wb = np.zeros((128, NWB), f32)
    pf = np.zeros((48, NPF), f32)
    x = np.asarray(I["x"], f32)[0, 0]
    idx = np.arange(TDN)[:, None] + np.arange(OFC)[None, :]
    eeg = x[1:17, WL - OFC:]
    wb[0:OFC, C_EEGT:C_EEGT + 16] = eeg.T
    wb[0:OFC, C_KAT:C_KAT + TDN] = x[0][idx].T
    wb[0:OFC, C_KBT:C_KBT + TDN] = x[17][idx].T
    wb[OFC, C_EEGT:C_EEGT + 58] = 1.0      # ones row for bias folds
    wb[0:16, C_EEG:C_EEG + OFC] = eeg

    s1 = 1.0 / math.sqrt(OFC)
    for bi, p in enumerate(("tdA", "tdB")):
        inw = np.asarray(I[f"{p}_in_w"], f32)
        inb = np.asarray(I[f"{p}_in_b"], f32)
        outw = np.asarray(I[f"{p}_out_w"], f32)
        outb = np.asarray(I[f"{p}_out_b"], f32)
        o = C_QKV_A if bi == 0 else C_QKV_B
        wb[0:OFC, o:o + 118] = inw[0:OFC].T * s1
        wb[0:OFC, o + 118:o + 236] = inw[OFC:2 * OFC].T
        wb[0:OFC, o + 236:o + 354] = inw[2 * OFC:3 * OFC].T
        wb[OFC, o:o + 118] = inb[0:OFC] * s1           # bq row
        wb[OFC, o + 118:o + 236] = inb[OFC:2 * OFC]    # bk row
        bv = inb[2 * OFC:3 * OFC]
        obr = bv @ outw.T + outb
        r = PF_OBR_A if bi == 0 else PF_OBR_B
        pf[0, r[0]:r[1]] = obr
        wb[0:OFC, (C_OB16_A if bi == 0 else C_OB16_B)] = 16.0 * obr
        wo = C_WO_A if bi == 0 else C_WO_B
        wot = C_WOT_A if bi == 0 else C_WOT_B
        wb[0:OFC, wo:wo + 118] = outw
        wb[0:OFC, wot:wot + 118] = outw.T
    wb[0:OFC, C_ONES118] = 1.0
    for i in range(4):
        wb[i, C_EE + 16 * i:C_EE + 16 * (i + 1)] = 1.0
    pf[0, PF_ID1] = 1.0

    cmw = np.asarray(I["cm_in_w"], f32)
    cmb = np.asarray(I["cm_in_b"], f32)
    cow = np.asarray(I["cm_out_w"], f32)
    cob = np.asarray(I["cm_out_b"], f32)
    pA = np.asarray(I["projA_w"], f32)[:, 0]
    pB = np.asarray(I["projB_w"], f32)[:, 0]
    s2 = 1.0 / math.sqrt(D_CM)
    wq, wk, wv = cmw[:, 0:16], cmw[:, 16:32], cmw[:, 32:48]
    bq, bv2 = cmb[:, 0:16], cmb[:, 32:48]
    wb[0:16, C_WQ1T:C_WQ1T + 16] = wq[1].T * s2
    wb[0:16, C_WQ2T:C_WQ2T + 16] = wq[2].T * s2
    wb[0:16, C_WK0T:C_WK0T + 16] = wk[0].T
    wb[0:16, C_WK3T:C_WK3T + 16] = wk[3].T
    wb[0:16, C_WV03:C_WV03 + 16] = wv[0].T
    wb[0:16, C_WV03 + 16:C_WV03 + 32] = wv[3].T
    wb[0:16, C_UQ0] = (wq[0] @ pA) * s2
    wb[0:16, C_BQ0] = bq[0] * s2
    wb[0:16, C_UQ3] = (wq[3] @ pB) * s2
    wb[0:16, C_BQ3] = bq[3] * s2
    wb[0:16, C_UK1] = wk[1] @ pA
    wb[0:16, C_UK2] = wk[2] @ pB
    wb[0, C_VV1:C_VV1 + 16] = wv[1] @ pA
    wb[0, C_VV2:C_VV2 + 16] = wv[2] @ pB
    pf[0, PF_KAP1] = (bq[1] * s2) @ (wk[1] @ pA)
    pf[0, PF_KAP2] = (bq[2] * s2) @ (wk[2] @ pB)

    convw = np.asarray(I["conv_w"], f32)
    obeff = np.stack([cow[i] @ bv2[i] + cob[i] for i in range(4)])
    const = np.asarray(I["conv_b"], f32).reshape(-1).copy()
    for i in range(4):
        for k in range(KS):
            wb[16 * i:16 * (i + 1),
               C_CONV + 40 * k + 10 * i:C_CONV + 40 * k + 10 * (i + 1)] = (
                convw[i, :, :, k] @ cow[i]).T
        const[10 * i:10 * (i + 1)] += np.einsum("ock,c->o", convw[i], obeff[i])
    pf[0:40, PF_CCONST] = const
    wb[0:40, C_FC1:C_FC1 + 40] = np.asarray(I["fc1_w"], f32).T
    wb[0:40, C_FC2:C_FC2 + 2] = np.asarray(I["fc2_w"], f32).T
    pf[0:40, PF_NEGB1] = -np.asarray(I["fc1_b"], f32)
    pf[0:2, PF_NEGB2] = -np.asarray(I["fc2_b"], f32)

    wbb = wb.astype(BF)
    return (np.ascontiguousarray(wbb[0:119, 0:1241]),
            np.ascontiguousarray(wbb[0:64, 1241:1960]),
            np.ascontiguousarray(pf))


def _emit(nc, tc, H, out_ap):
    AF = mybir.ActivationFunctionType
    ALU = mybir.AluOpType
    X = mybir.AxisListType.X

    ctx = ExitStack()
    consts = ctx.enter_context(tc.tile_pool(name="consts", bufs=1))
    work = ctx.enter_context(tc.tile_pool(name="work", bufs=1))
    psum = ctx.enter_context(tc.tile_pool(name="psum", bufs=1, space="PSUM"))

    def dram_ap(handle, dims):
        return bass.AP(tensor=handle, offset=0, ap=[list(d) for d in dims])

    def pst(shape, nm, tag, bufs=2):
        return psum.tile(shape, F32, name=nm, tag=tag, bufs=bufs)

    # --------- direct 1-hop HBM loads (descriptor budget fits the bytes) ---
    wb = consts.tile([128, NWB], BF16, name="wb")
    nc.sync.dma_start(out=wb[0:119, 0:1241],
                      in_=dram_ap(H["wbx"], [(1241, 119), (1, 1241)]))
    nc.scalar.dma_start(out=wb[0:64, 1241:1960],
                        in_=dram_ap(H["wlo"], [(719, 64), (1, 719)]))
    pf = consts.tile([48, NPF], F32, name="pf")
    nc.scalar.dma_start(out=pf[:, :],
                        in_=dram_ap(H["pfx"], [(NPF, 48), (1, NPF)]))

    id118 = consts.tile([OFC, OFC], F32, name="id118")
    make_identity(nc, id118)
    vpads = work.tile([OFC, 256], BF16, name="vpads")

    eegT = wb[0:OFC, C_EEGT:C_EEGT + 16]
    eegT1 = wb[0:OFC + 1, C_EEGT:C_EEGT + 16]       # with ones row
    kT = {"A": wb[0:OFC, C_KAT:C_KAT + TDN], "B": wb[0:OFC, C_KBT:C_KBT + TDN]}
    kT1 = {"A": wb[0:OFC + 1, C_KAT:C_KAT + TDN],
           "B": wb[0:OFC + 1, C_KBT:C_KBT + TDN]}
    eeg = wb[0:16, C_EEG:C_EEG + OFC]

    s1 = {"A": {}, "B": {}}
    tagm = {"A": "tA", "B": "tB"}
    qkvo = {"A": C_QKV_A, "B": C_QKV_B}
    woN = {"A": wb[0:OFC, C_WO_A:C_WO_A + 118], "B": wb[0:OFC, C_WO_B:C_WO_B + 118]}
    woT = {"A": wb[0:OFC, C_WOT_A:C_WOT_A + 118],
           "B": wb[0:OFC, C_WOT_B:C_WOT_B + 118]}
    ob16 = {"A": wb[0:OFC, C_OB16_A:C_OB16_A + 1],
            "B": wb[0:OFC, C_OB16_B:C_OB16_B + 1]}
    obrr = {"A": pf[0:1, PF_OBR_A[0]:PF_OBR_A[1]],
            "B": pf[0:1, PF_OBR_B[0]:PF_OBR_B[1]]}
    kapc = {1: pf[0:1, PF_KAP1:PF_KAP1 + 1], 2: pf[0:1, PF_KAP2:PF_KAP2 + 1]}
    id1 = pf[0:1, PF_ID1:PF_ID1 + 1]
    s_ps = pst([1, 32], "s_ps", "tS", bufs=1)  # A cols 0:16, B cols 16:32

    def ps1(br, shape, nm):
        return pst(shape, f"{nm}_{br}", tagm[br])

    def qk_mms(br):
        d = s1[br]
        o = qkvo[br]
        d["qpT_ps"] = ps1(br, [OFC, 16], "qpT")
        nc.tensor.matmul(d["qpT_ps"][:, :], wb[0:OFC + 1, o:o + 118], eegT1)
        d["kpT_ps"] = ps1(br, [OFC, TDN], "kpT")
        nc.tensor.matmul(d["kpT_ps"][:, :], wb[0:OFC + 1, o + 118:o + 236], kT1[br])

    def qk_cps(br):
        d = s1[br]
        d["qpT"] = work.tile([OFC, 16], BF16, name=f"qpT_{br}")
        nc.vector.tensor_copy(d["qpT"][:, :], d["qpT_ps"][:, :])
        d["kpT"] = work.tile([OFC, TDN], BF16, name=f"kpT_{br}")
        nc.vector.tensor_copy(d["kpT"][:, :], d["kpT_ps"][:, :])

    def vphT_mm(br):
        d = s1[br]
        o = qkvo[br]
        d["vphT_ps"] = ps1(br, [OFC, TDN], "vphT")
        nc.tensor.matmul(d["vphT_ps"][:, :], wb[0:OFC, o + 236:o + 354], kT[br])

    def vphT_cp(br):
        d = s1[br]
        d["vphT"] = work.tile([OFC, TDN], BF16, name=f"vphT_{br}")
        nc.scalar.copy(d["vphT"][:, :], d["vphT_ps"][:, :])

    def s_mm(br):
        d = s1[br]
        d["S_ps"] = ps1(br, [16, TDN], "S")
        nc.tensor.matmul(d["S_ps"][:, :], d["qpT"][:, :], d["kpT"][:, :])

    def csel_mm(br):
        off = 0 if br == "A" else 16
        nc.tensor.matmul(s_ps[0:1, off:off + 16], ob16[br], eegT,
                         start=True, stop=False)

    def gt_mm(br):
        d = s1[br]
        d["GT_ps"] = ps1(br, [OFC, 16], "GT")
        nc.tensor.matmul(d["GT_ps"][:, :], woN[br], eegT)

    def gt_cp(br):
        d = s1[br]
        d["GT"] = work.tile([OFC, 16], BF16, name=f"GT_{br}")
        nc.scalar.copy(d["GT"][:, :], d["GT_ps"][:, :])

    def softmax1(br):
        d = s1[br]
        d["P"] = work.tile([16, TDN], F32, name=f"P_{br}")
        d["rowsum"] = work.tile([16, 1], F32, name=f"rowsum_{br}")
        nc.scalar.activation(d["P"][:, :], d["S_ps"][:, :], AF.Exp,
                             scale=1.0, accum_out=d["rowsum"][:, :])
        d["rinv"] = work.tile([16, 1], F32, name=f"rinv_{br}")
        nc.vector.reciprocal(d["rinv"][:, :], d["rowsum"][:, :])

    def ncol_mm(br):
        d = s1[br]
        d["ncol_ps"] = ps1(br, [TDN, 1], "ncol")
        nc.tensor.matmul(d["ncol_ps"][:, :], d["P"][:, :], d["rinv"][:, :])

    def ncol_cp(br):
        d = s1[br]
        d["ncol"] = work.tile([TDN, 1], BF16, name=f"ncol_{br}")
        nc.vector.tensor_copy(d["ncol"][:, :], d["ncol_ps"][:, :])

    def m2_mm(br):
        d = s1[br]
        d["M2_ps"] = ps1(br, [TDN, 16], "M2")
        nc.tensor.matmul(d["M2_ps"][:, :], d["vphT"][:, :], d["GT"][:, :])

    def m2_cp(br):
        d = s1[br]
        d["M2"] = work.tile([TDN, 16], BF16, name=f"M2_{br}")
        nc.scalar.copy(d["M2"][:, :], d["M2_ps"][:, :])

    def zw_mm(br):
        d = s1[br]
        d["Zw_ps"] = ps1(br, [TDN, OFC], "Zw")
        nc.tensor.matmul(d["Zw_ps"][:, :], d["vphT"][:, :], woT[br])

    def zw_cp(br):
        d = s1[br]
        d["Zw"] = work.tile([TDN, OFC], BF16, name=f"Zw_{br}")
        nc.scalar.copy(d["Zw"][:, :], d["Zw_ps"][:, :])

    def s_mm2(br):
        d = s1[br]
        off = 0 if br == "A" else 16
        nc.tensor.matmul(s_ps[0:1, off:off + 16], d["ncol"][:, :], d["M2"][:, :],
                         start=False, stop=True)

    def sel_post(br):
        d = s1[br]
        off = 0 if br == "A" else 16
        d["m"] = work.tile([1, 1], F32, name=f"m_{br}")
        nc.vector.reduce_max(d["m"][:, :], s_ps[0:1, off:off + 16], axis=X)
        d["oh"] = work.tile([1, 16], F32, name=f"oh_{br}")
        nc.vector.tensor_scalar(d["oh"][:, :], s_ps[0:1, off:off + 16],
                                d["m"][:, :], None, op0=ALU.is_equal)

    def oht_mm(br):
        d = s1[br]
        d["ohT_ps"] = ps1(br, [16, 1], "ohT")
        nc.tensor.transpose(d["ohT_ps"][:, :], d["oh"][:, :], id1)

    def rh_cp(br):
        d = s1[br]
        d["rh"] = work.tile([16, 1], F32, name=f"rh_{br}")
        nc.vector.tensor_mul(d["rh"][:, :], d["ohT_ps"][:, :], d["rinv"][:, :])

    def nsel_mm(br):
        d = s1[br]
        d["nsel_ps"] = ps1(br, [TDN, 1], "nsel")
        nc.tensor.matmul(d["nsel_ps"][:, :], d["P"][:, :], d["rh"][:, :])

    def nsel_cp(br):
        d = s1[br]
        d["nsel"] = work.tile([TDN, 1], BF16, name=f"nsel_{br}")
        nc.vector.tensor_copy(d["nsel"][:, :], d["nsel_ps"][:, :])

    def row_mm(br):
        d = s1[br]
        d["row_ps"] = ps1(br, [1, OFC], "row")
        nc.tensor.matmul(d["row_ps"][:, :], d["nsel"][:, :], d["Zw"][:, :])

    def row_cp(br):
        d = s1[br]
        d["row"] = work.tile([1, OFC], BF16, name=f"row_{br}")
        nc.vector.tensor_add(d["row"][:, :], d["row_ps"][:, :], obrr[br])

    e = {}

    def eproj(nm, col):
        ps = pst([16, OFC], f"{nm}_ps", "tE")
        nc.tensor.matmul(ps[:, :], wb[0:16, col:col + 16], eeg)
        e[nm + "_ps"] = ps

    def eproj_cp(nm):
        t = work.tile([16, OFC], BF16, name=nm)
        nc.scalar.copy(t[:, :], e[nm + "_ps"][:, :])
        e[nm] = t

    def factor_mm(nm, lhsT, rhs, shape):
        ps = pst(shape, f"{nm}_ps", "tE")
        nc.tensor.matmul(ps[:, :], lhsT, rhs)
        e[nm + "_ps"] = ps

    # ---------------- emission order ----------------------------------------
    qk_mms("A")
    qk_mms("B")
    qk_cps("A")
    nc.vector.memset(vpads[:, :], 0.0)
    vphT_mm("A")
    vphT_mm("B")
    qk_cps("B")
    s_mm("A")
    csel_mm("A")
    vphT_cp("A")
    s_mm("B")
    csel_mm("B")
    vphT_cp("B")
    softmax1("A")
    gt_mm("A")
    gt_mm("B")
    softmax1("B")
    gt_cp("A")
    gt_cp("B")
    ncol_mm("A")
    m2_mm("A")
    ncol_cp("A")
    ncol_mm("B")
    m2_mm("B")
    ncol_cp("B")
    m2_cp("A")
    m2_cp("B")
    eproj("qp1T", C_WQ1T)
    zw_mm("A")
    s_mm2("A")
    eproj_cp("qp1T")
    zw_mm("B")
    sel_post("A")
    zw_cp("A")
    eproj("kp0T", C_WK0T)
    s_mm2("B")
    oht_mm("A")
    sel_post("B")
    rh_cp("A")
    zw_cp("B")
    eproj_cp("kp0T")
    nsel_mm("A")
    oht_mm("B")
    nsel_cp("A")
    rh_cp("B")
    eproj("qp2T", C_WQ2T)
    nsel_mm("B")
    eproj_cp("qp2T")
    row_mm("A")
    nsel_cp("B")
    row_cp("A")
    eproj("kp3T", C_WK3T)
    row_mm("B")
    eproj_cp("kp3T")
    row_cp("B")
    rowS = {"A": s1["A"]["row"], "B": s1["B"]["row"]}
    # rank-1 factors + vp0/vp3 (eeg-side): independent of rowA/rowB, emitted
    # before the row-dependent matmuls so they fill select-phase gaps
    vp03_ps = pst([OFC, 32], "vp03_ps", "tE")
    nc.tensor.matmul(vp03_ps[:, :], eeg, wb[0:16, C_WV03:C_WV03 + 32])
    nc.vector.tensor_copy(vpads[:, 0:16], vp03_ps[:, 0:16])
    nc.vector.tensor_copy(vpads[:, 240:256], vp03_ps[:, 16:32])
    factor_mm("g0", wb[0:16, C_UQ0:C_UQ0 + 1], e["kp0T"][:, :], [1, OFC])
    g0 = work.tile([1, OFC], BF16, name="g0")
    nc.vector.tensor_copy(g0[:, :], e["g0_ps"][:, :])
    factor_mm("c0", e["kp0T"][:, :], wb[0:16, C_BQ0:C_BQ0 + 1], [OFC, 1])
    c0 = work.tile([OFC, 1], F32, name="c0")
    nc.scalar.copy(c0[:, :], e["c0_ps"][:, :])
    factor_mm("h1", wb[0:16, C_UK1:C_UK1 + 1], e["qp1T"][:, :], [1, OFC])
    h1 = work.tile([1, OFC], BF16, name="h1")
    nc.vector.tensor_scalar_add(h1[:, :], e["h1_ps"][:, :], kapc[1])
    factor_mm("g3", wb[0:16, C_UQ3:C_UQ3 + 1], e["kp3T"][:, :], [1, OFC])
    g3 = work.tile([1, OFC], BF16, name="g3")
    nc.vector.tensor_copy(g3[:, :], e["g3_ps"][:, :])
    factor_mm("c3", e["kp3T"][:, :], wb[0:16, C_BQ3:C_BQ3 + 1], [OFC, 1])
    c3 = work.tile([OFC, 1], F32, name="c3")
    nc.scalar.copy(c3[:, :], e["c3_ps"][:, :])
    factor_mm("h2", wb[0:16, C_UK2:C_UK2 + 1], e["qp2T"][:, :], [1, OFC])
    h2 = work.tile([1, OFC], BF16, name="h2")
    nc.vector.tensor_scalar_add(h2[:, :], e["h2_ps"][:, :], kapc[2])

    # rowA/rowB dependent
    vp1_ps = pst([OFC, 16], "vp1_ps", "tS", bufs=1)
    nc.tensor.matmul(vp1_ps[:, :], rowS["A"][:, :], wb[0:1, C_VV1:C_VV1 + 16])
    nc.vector.tensor_copy(vpads[:, 80:96], vp1_ps[:, :])
    pt_ps = [None] * 4
    pt_ps[0] = pst([OFC, OFC], "pt0_ps", "tA")
    nc.tensor.matmul(pt_ps[0][:, :], g0[:, :], rowS["A"][:, :])
    pt_ps[1] = pst([OFC, OFC], "pt1_ps", "tA")
    nc.tensor.matmul(pt_ps[1][:, :], rowS["A"][:, :], h1[:, :])
    vp2_ps = pst([OFC, 16], "vp2_ps", "tS", bufs=1)
    nc.tensor.matmul(vp2_ps[:, :], rowS["B"][:, :], wb[0:1, C_VV2:C_VV2 + 16])
    nc.vector.tensor_copy(vpads[:, 160:176], vp2_ps[:, :])
    pt_ps[2] = pst([OFC, OFC], "pt2_ps", "tB")
    nc.tensor.matmul(pt_ps[2][:, :], rowS["B"][:, :], h2[:, :])
    pt_ps[3] = pst([OFC, OFC], "pt3_ps", "tB")
    nc.tensor.matmul(pt_ps[3][:, :], g3[:, :], rowS["B"][:, :])

    ptall = work.tile([OFC, 4 * OFC], BF16, name="ptall")
    biases = [c0, None, None, c3]
    for i in range(4):
        b = biases[i]
        nc.scalar.activation(ptall[:, OFC * i:OFC * (i + 1)], pt_ps[i][:, :],
                             AF.Exp, bias=(b[:, :] if b is not None else 0.0),
                             scale=1.0)

    ztall_ps = pst([64, OFC], "ztall_ps", "tZ", bufs=1)
    rs_ps = pst([OFC, 4], "rs_ps", "tS", bufs=1)
    ones118 = wb[0:OFC, C_ONES118:C_ONES118 + 1]
    for i in range(4):
        nc.tensor.matmul(rs_ps[:, i:i + 1],
                         ptall[:, OFC * i:OFC * (i + 1)], ones118)
        nc.tensor.matmul(ztall_ps[:, :], vpads[:, 64 * i:64 * (i + 1)],
                         ptall[:, OFC * i:OFC * (i + 1)],
                         start=(i == 0), stop=(i == 3))
    rinv_col = work.tile([OFC, 4], F32, name="rinv_col")
    nc.vector.reciprocal(rinv_col[:, :], rs_ps[:, :])
    r4t_ps = pst([4, OFC], "r4t_ps", "tE")
    nc.tensor.transpose(r4t_ps[:, :], rinv_col[:, :], id118[:, :])
    r4 = work.tile([4, OFC], BF16, name="r4")
    nc.vector.tensor_copy(r4[:, :], r4t_ps[:, :])
    m_ps = pst([64, OFC], "m_ps", "tE")
    nc.tensor.matmul(m_ps[:, :], wb[0:4, C_EE:C_EE + 64], r4[:, :])
    m_sb = work.tile([64, OFC], F32, name="m_sb")
    nc.vector.tensor_copy(m_sb[:, :], m_ps[:, :])
    ztn = work.tile([64, OFC], BF16, name="ztn")
    nc.vector.tensor_mul(ztn[:, :], ztall_ps[:, :], m_sb[:, :])

    # ---------------- conv + head ------------------------------------------
    y_ps = pst([4 * C_OUT, NCONV], "y_ps", "tZ", bufs=1)
    for k in range(KS):
        nc.tensor.matmul(y_ps[:, :],
                         wb[0:64, C_CONV + 40 * k:C_CONV + 40 * (k + 1)],
                         ztn[:, k:k + NCONV], start=(k == 0), stop=(k == KS - 1))
    relu = work.tile([4 * C_OUT, NCONV], F32, name="relu")
    nc.scalar.activation(relu[:, :], y_ps[:, :], AF.Relu,
                         bias=pf[0:40, PF_CCONST:PF_CCONST + 1], scale=1.0)
    feat = work.tile([4 * C_OUT, 1], BF16, name="feat")
    nc.vector.reduce_max(feat[:, :], relu[:, :], axis=X)

    h_ps = pst([40, 1], "h_ps", "tZ", bufs=1)
    nc.tensor.matmul(h_ps[:, :], wb[0:40, C_FC1:C_FC1 + 40], feat[:, :])
    eh = work.tile([40, 1], F32, name="eh")
    nc.scalar.activation(eh[:, :], h_ps[:, :], AF.Exp,
                         bias=pf[0:40, PF_NEGB1:PF_NEGB1 + 1], scale=-1.0)
    eh1 = work.tile([40, 1], F32, name="eh1")
    nc.vector.tensor_scalar(eh1[:, :], eh[:, :], 1.0, None, op0=ALU.add)
    hsb = work.tile([40, 1], BF16, name="hsb")
    with nc.allow_low_precision(reason="bf16 operand for the 2x40 head matmul"):
        nc.vector.reciprocal(hsb[:, :], eh1[:, :])
    o_ps = pst([2, 1], "o_ps", "tZ", bufs=1)
    nc.tensor.matmul(o_ps[:, :], wb[0:40, C_FC2:C_FC2 + 2], hsb[:, :])
    eo = work.tile([2, 1], F32, name="eo")
    nc.scalar.activation(eo[:, :], o_ps[:, :], AF.Exp,
                         bias=pf[0:2, PF_NEGB2:PF_NEGB2 + 1], scale=-1.0)
    eo1 = work.tile([2, 1], F32, name="eo1")
    nc.vector.tensor_scalar(eo1[:, :], eo[:, :], 1.0, None, op0=ALU.add)
    res = work.tile([2, 1], F32, name="res")
    nc.vector.reciprocal(res[:, :], eo1[:, :])
    nc.sync.dma_start(out=out_ap, in_=res[:, :])
    ctx.close()


_CACHE = {}


def build():
    if "nc" in _CACHE:
        return _CACHE["nc"]
    nc = bacc.Bacc("TRN2", target_bir_lowering=False, debug=False,
                   num_devices=N_CORES)
    H = {
        "wbx": nc.dram_tensor("wbx", [119, 1241], BF16, kind="ExternalInput"),
        "wlo": nc.dram_tensor("wlo", [64, 719], BF16, kind="ExternalInput"),
        "pfx": nc.dram_tensor("pfx", [48, NPF], F32, kind="ExternalInput"),
    }
    out_t = nc.dram_tensor("out", [1, 2], F32, kind="ExternalOutput")
    with tile.TileContext(nc) as tc:
        _emit(nc, tc, H, out_t.ap())
    nc.compile()
    _CACHE["nc"] = nc
    return nc


def pack_inputs(inputs):
    wbx, wlo, pfx = host_pack(inputs)
    return {"wbx": wbx, "wlo": wlo, "pfx": pfx}


def kernel(**inputs):
    in_map = pack_inputs(inputs)
    nc = build()
    res = run_bass_kernel_spmd(nc, [in_map] * N_CORES,
                               core_ids=list(range(N_CORES)))
    return res.results[0]["out"]


# revision 14
# speedup vs baseline: 1.4692x; 1.0215x over previous
"""Trainium2 Bass/Tile kernel for nn_CNN_77077483094746 (v4).

Single tiny sample (x: [1,1,18,140]) -> (1,2); whole forward on one core,
SPMD on 8 cores with identical inputs, output from core 0.

Measured facts driving the design (NTFF traces on this part):
- HBM->SBUF DMA pays ~2us PER DESCRIPTOR per engine (not pipelined within an
  engine); SBUF->SBUF descriptors cost ~0.45us/engine. So: ONE bf16 HBM load
  shaped [16, M] (16 fat descriptors), then two row-half SBUF reshapes (64
  descriptors each) running on both HWDGE queues in parallel.
- qp/kp biases are folded into the projection matmuls as a K=119 contraction
  (bias row appended to the weights, ones row appended to eegT/kAT/kBT) so
  no f32 bias columns are needed early.
- DVE ops on single-partition rows serialize per lane (a [1,118] reciprocal
  costs ~880ns). The softmax normalizer is computed as [118,4] columns (one
  ~60ns reciprocal), PE-transposed to [4,118], spread to a [64,118] mask by
  one indicator matmul.
- All weight reformatting on HOST (pre-transposed/scaled/folded, bf16).
- wA/wB are rank-1, so every stage-2 score matrix is rank-1, materialized
  TRANSPOSED by one K=1 outer-product matmul; no [118,118] transposes.
- Wo + output-side biases folded into conv weights / relu bias on host.
- Stage-1 select: s = ncol^T M2 + csel with M2/Zw/GT attention-independent.
- attn max-subtraction dropped (|S| < 2 measured); sigmoid via exp table.
"""
import math
from contextlib import ExitStack

import numpy as np
import ml_dtypes

import concourse.bass as bass
import concourse.mybir as mybir
import concourse.tile as tile
from concourse import bacc
from concourse.bass_utils import run_bass_kernel_spmd
from concourse.masks import make_identity

WL = 140
OFC = 118
TDN = 21
D_CM = 16
N_BR = 4
C_OUT = 10
KS = 9
NCONV = OFC - KS + 1
F32 = mybir.dt.float32
BF16 = mybir.dt.bfloat16
N_CORES = 8
BF = ml_dtypes.bfloat16

INPUT_SPECS = {
    "x": (1, 1, 18, WL),
    "tdA_in_w": (3 * OFC, OFC), "tdA_in_b": (3 * OFC,),
    "tdA_out_w": (OFC, OFC), "tdA_out_b": (OFC,),
    "tdB_in_w": (3 * OFC, OFC), "tdB_in_b": (3 * OFC,),
    "tdB_out_w": (OFC, OFC), "tdB_out_b": (OFC,),
    "cm_in_w": (N_BR, 3 * D_CM, D_CM), "cm_in_b": (N_BR, 3 * D_CM),
    "cm_out_w": (N_BR, D_CM, D_CM), "cm_out_b": (N_BR, D_CM),
    "projA_w": (16, 1), "projB_w": (16, 1),
    "conv_w": (N_BR, C_OUT, 16, KS), "conv_b": (N_BR, C_OUT),
    "fc1_w": (40, 40), "fc1_b": (40,),
    "fc2_w": (2, 40), "fc2_b": (2,),
}

# ---------------- wb bf16 [128, 1960] column layout -------------------------
C_EEGT = 0        # [0:119, 0:16]   eegT; row 118 = 1.0 (bias fold)
C_KAT = 16        # [0:119, 16:37]  kA^T; row 118 = 1.0
C_KBT = 37        # [0:119, 37:58]  kB^T; row 118 = 1.0
C_QKV_A = 58      # [0:119, 58:412] WqT|WkT|WvT; row 118 = bq|bk|0
C_QKV_B = 412
C_OB16_A = 766    # [118,1] 16*obr
C_OB16_B = 767
C_WO_A = 768      # [118,118] Wo as stored [o,c]
C_WOT_A = 886     # [118,118] Wo^T [c,o]
C_WO_B = 1004
C_WOT_B = 1122
C_ONES118 = 1240  # [118,1] ones
C_CONV = 1241     # rows 0:64, 9 slices of 40
C_FC1 = 1601      # rows 0:40 [40,40]
C_FC2 = 1641      # rows 0:40 [40,2]
C_EEG = 1643      # rows 0:16 [16,118] eeg natural
C_WQ1T = 1761     # rows 0:16 [16,16]
C_WQ2T = 1777
C_WK0T = 1793
C_WK3T = 1809
C_WV03 = 1825     # rows 0:16 [16,32]
C_UQ0, C_BQ0, C_UQ3, C_BQ3, C_UK1, C_UK2 = 1857, 1858, 1859, 1860, 1861, 1862
C_VV1 = 1863      # row 0 [1,16]
C_VV2 = 1879
C_EE = 1895       # rows 0:4 [4,64]
NWB = 1960
# pf f32 [48, 242]
PF_NEGB1, PF_NEGB2, PF_CCONST = 0, 1, 2
PF_KAP1, PF_KAP2, PF_ID1 = 3, 4, 5    # row-0 scalars
PF_OBR_A = (6, 124)                   # row 0 [1,118]
PF_OBR_B = (124, 242)
NPF = 242


def host_pack(I):
    """Weight-only reformatting + x marshaling -> ([16,*] bf16, [16,*] f32)."""
    f32 = np.float32
    wb = np.zeros((128, NWB), f32)
    pf = np.zeros((48, NPF), f32)
    x = np.asarray(I["x"], f32)[0, 0]
    idx = np.arange(TDN)[:, None] + np.arange(OFC)[None, :]
    eeg = x[1:17, WL - OFC:]
    wb[0:OFC, C_EEGT:C_EEGT + 16] = eeg.T
    wb[0:OFC, C_KAT:C_KAT + TDN] = x[0][idx].T
    wb[0:OFC, C_KBT:C_KBT + TDN] = x[17][idx].T
    wb[OFC, C_EEGT:C_EEGT + 58] = 1.0      # ones row for bias folds
    wb[0:16, C_EEG:C_EEG + OFC] = eeg

    s1 = 1.0 / math.sqrt(OFC)
    for bi, p in enumerate(("tdA", "tdB")):
        inw = np.asarray(I[f"{p}_in_w"], f32)
        inb = np.asarray(I[f"{p}_in_b"], f32)
        outw = np.asarray(I[f"{p}_out_w"], f32)
        outb = np.asarray(I[f"{p}_out_b"], f32)
        o = C_QKV_A if bi == 0 else C_QKV_B
        wb[0:OFC, o:o + 118] = inw[0:OFC].T * s1
        wb[0:OFC, o + 118:o + 236] = inw[OFC:2 * OFC].T
        wb[0:OFC, o + 236:o + 354] = inw[2 * OFC:3 * OFC].T
        wb[OFC, o:o + 118] = inb[0:OFC] * s1           # bq row
        wb[OFC, o + 118:o + 236] = inb[OFC:2 * OFC]    # bk row
        bv = inb[2 * OFC:3 * OFC]
        obr = bv @ outw.T + outb
        r = PF_OBR_A if bi == 0 else PF_OBR_B
        pf[0, r[0]:r[1]] = obr
        wb[0:OFC, (C_OB16_A if bi == 0 else C_OB16_B)] = 16.0 * obr
        wo = C_WO_A if bi == 0 else C_WO_B
        wot = C_WOT_A if bi == 0 else C_WOT_B
        wb[0:OFC, wo:wo + 118] = outw
        wb[0:OFC, wot:wot + 118] = outw.T
    wb[0:OFC, C_ONES118] = 1.0
    for i in range(4):
        wb[i, C_EE + 16 * i:C_EE + 16 * (i + 1)] = 1.0
    pf[0, PF_ID1] = 1.0

    cmw = np.asarray(I["cm_in_w"], f32)
    cmb = np.asarray(I["cm_in_b"], f32)
    cow = np.asarray(I["cm_out_w"], f32)
    cob = np.asarray(I["cm_out_b"], f32)
    pA = np.asarray(I["projA_w"], f32)[:, 0]
    pB = np.asarray(I["projB_w"], f32)[:, 0]
    s2 = 1.0 / math.sqrt(D_CM)
    wq, wk, wv = cmw[:, 0:16], cmw[:, 16:32], cmw[:, 32:48]
    bq, bv2 = cmb[:, 0:16], cmb[:, 32:48]
    wb[0:16, C_WQ1T:C_WQ1T + 16] = wq[1].T * s2
    wb[0:16, C_WQ2T:C_WQ2T + 16] = wq[2].T * s2
    wb[0:16, C_WK0T:C_WK0T + 16] = wk[0].T
    wb[0:16, C_WK3T:C_WK3T + 16] = wk[3].T
    wb[0:16, C_WV03:C_WV03 + 16] = wv[0].T
    wb[0:16, C_WV03 + 16:C_WV03 + 32] = wv[3].T
    wb[0:16, C_UQ0] = (wq[0] @ pA) * s2
    wb[0:16, C_BQ0] = bq[0] * s2
    wb[0:16, C_UQ3] = (wq[3] @ pB) * s2
    wb[0:16, C_BQ3] = bq[3] * s2
    wb[0:16, C_UK1] = wk[1] @ pA
    wb[0:16, C_UK2] = wk[2] @ pB
    wb[0, C_VV1:C_VV1 + 16] = wv[1] @ pA
    wb[0, C_VV2:C_VV2 + 16] = wv[2] @ pB
    pf[0, PF_KAP1] = (bq[1] * s2) @ (wk[1] @ pA)
    pf[0, PF_KAP2] = (bq[2] * s2) @ (wk[2] @ pB)

    convw = np.asarray(I["conv_w"], f32)
    obeff = np.stack([cow[i] @ bv2[i] + cob[i] for i in range(4)])
    const = np.asarray(I["conv_b"], f32).reshape(-1).copy()
    for i in range(4):
        for k in range(KS):
            wb[16 * i:16 * (i + 1),
               C_CONV + 40 * k + 10 * i:C_CONV + 40 * k + 10 * (i + 1)] = (
                convw[i, :, :, k] @ cow[i]).T
        const[10 * i:10 * (i + 1)] += np.einsum("ock,c->o", convw[i], obeff[i])
    pf[0:40, PF_CCONST] = const
    wb[0:40, C_FC1:C_FC1 + 40] = np.asarray(I["fc1_w"], f32).T
    wb[0:40, C_FC2:C_FC2 + 2] = np.asarray(I["fc2_w"], f32).T
    pf[0:40, PF_NEGB1] = -np.asarray(I["fc1_b"], f32)
    pf[0:2, PF_NEGB2] = -np.asarray(I["fc2_b"], f32)

    wbb = wb.astype(BF)
    return (np.ascontiguousarray(wbb[0:119, 0:1241]),
            np.ascontiguousarray(wbb[0:64, 1241:1960]),
            np.ascontiguousarray(pf))


def _emit(nc, tc, H, out_ap):
    AF = mybir.ActivationFunctionType
    ALU = mybir.AluOpType
    X = mybir.AxisListType.X

    ctx = ExitStack()
    consts = ctx.enter_context(tc.tile_pool(name="consts", bufs=1))
    work = ctx.enter_context(tc.tile_pool(name="work", bufs=1))
    psum = ctx.enter_context(tc.tile_pool(name="psum", bufs=1, space="PSUM"))

    def dram_ap(handle, dims):
        return bass.AP(tensor=handle, offset=0, ap=[list(d) for d in dims])

    def pst(shape, nm, tag, bufs=2):
        return psum.tile(shape, F32, name=nm, tag=tag, bufs=bufs)

    # --------- direct 1-hop HBM loads (descriptor budget fits the bytes) ---
    wb = consts.tile([128, NWB], BF16, name="wb")
    nc.sync.dma_start(out=wb[0:119, 0:620],
                      in_=bass.AP(tensor=H["wbx"], offset=0,
                                  ap=[[1241, 119], [1, 620]]))
    nc.gpsimd.dma_start(out=wb[0:119, 620:1241],
                        in_=bass.AP(tensor=H["wbx"], offset=620,
                                    ap=[[1241, 119], [1, 621]]))
    nc.scalar.dma_start(out=wb[0:64, 1241:1960],
                        in_=dram_ap(H["wlo"], [(719, 64), (1, 719)]))
    pf = consts.tile([48, NPF], F32, name="pf")
    nc.scalar.dma_start(out=pf[:, :],
                        in_=dram_ap(H["pfx"], [(NPF, 48), (1, NPF)]))

    id118 = consts.tile([OFC, OFC], F32, name="id118")
    make_identity(nc, id118)
    vpads = work.tile([OFC, 256], BF16, name="vpads")

    eegT = wb[0:OFC, C_EEGT:C_EEGT + 16]
    eegT1 = wb[0:OFC + 1, C_EEGT:C_EEGT + 16]       # with ones row
    kT = {"A": wb[0:OFC, C_KAT:C_KAT + TDN], "B": wb[0:OFC, C_KBT:C_KBT + TDN]}
    kT1 = {"A": wb[0:OFC + 1, C_KAT:C_KAT + TDN],
           "B": wb[0:OFC + 1, C_KBT:C_KBT + TDN]}
    eeg = wb[0:16, C_EEG:C_EEG + OFC]

    s1 = {"A": {}, "B": {}}
    tagm = {"A": "tA", "B": "tB"}
    qkvo = {"A": C_QKV_A, "B": C_QKV_B}
    woN = {"A": wb[0:OFC, C_WO_A:C_WO_A + 118], "B": wb[0:OFC, C_WO_B:C_WO_B + 118]}
    woT = {"A": wb[0:OFC, C_WOT_A:C_WOT_A + 118],
           "B": wb[0:OFC, C_WOT_B:C_WOT_B + 118]}
    ob16 = {"A": wb[0:OFC, C_OB16_A:C_OB16_A + 1],
            "B": wb[0:OFC, C_OB16_B:C_OB16_B + 1]}
    obrr = {"A": pf[0:1, PF_OBR_A[0]:PF_OBR_A[1]],
            "B": pf[0:1, PF_OBR_B[0]:PF_OBR_B[1]]}
    kapc = {1: pf[0:1, PF_KAP1:PF_KAP1 + 1], 2: pf[0:1, PF_KAP2:PF_KAP2 + 1]}
    id1 = pf[0:1, PF_ID1:PF_ID1 + 1]
    s_ps = pst([1, 32], "s_ps", "tS", bufs=1)  # A cols 0:16, B cols 16:32

    def ps1(br, shape, nm):
        return pst(shape, f"{nm}_{br}", tagm[br])

    def qk_mms(br):
        d = s1[br]
        o = qkvo[br]
        d["qpT_ps"] = ps1(br, [OFC, 16], "qpT")
        nc.tensor.matmul(d["qpT_ps"][:, :], wb[0:OFC + 1, o:o + 118], eegT1)
        d["kpT_ps"] = ps1(br, [OFC, TDN], "kpT")
        nc.tensor.matmul(d["kpT_ps"][:, :], wb[0:OFC + 1, o + 118:o + 236], kT1[br])

    def qk_cps(br):
        d = s1[br]
        d["qpT"] = work.tile([OFC, 16], BF16, name=f"qpT_{br}")
        nc.vector.tensor_copy(d["qpT"][:, :], d["qpT_ps"][:, :])
        d["kpT"] = work.tile([OFC, TDN], BF16, name=f"kpT_{br}")
        nc.vector.tensor_copy(d["kpT"][:, :], d["kpT_ps"][:, :])

    def vphT_mm(br):
        d = s1[br]
        o = qkvo[br]
        d["vphT_ps"] = ps1(br, [OFC, TDN], "vphT")
        nc.tensor.matmul(d["vphT_ps"][:, :], wb[0:OFC, o + 236:o + 354], kT[br])

    def vphT_cp(br):
        d = s1[br]
        d["vphT"] = work.tile([OFC, TDN], BF16, name=f"vphT_{br}")
        nc.scalar.copy(d["vphT"][:, :], d["vphT_ps"][:, :])

    def s_mm(br):
        d = s1[br]
        d["S_ps"] = ps1(br, [16, TDN], "S")
        nc.tensor.matmul(d["S_ps"][:, :], d["qpT"][:, :], d["kpT"][:, :])

    def csel_mm(br):
        off = 0 if br == "A" else 16
        nc.tensor.matmul(s_ps[0:1, off:off + 16], ob16[br], eegT,
                         start=True, stop=False)

    def gt_mm(br):
        d = s1[br]
        d["GT_ps"] = ps1(br, [OFC, 16], "GT")
        nc.tensor.matmul(d["GT_ps"][:, :], woN[br], eegT)

    def gt_cp(br):
        d = s1[br]
        d["GT"] = work.tile([OFC, 16], BF16, name=f"GT_{br}")
        nc.scalar.copy(d["GT"][:, :], d["GT_ps"][:, :])

    def softmax1(br):
        d = s1[br]
        d["P"] = work.tile([16, TDN], F32, name=f"P_{br}")
        d["rowsum"] = work.tile([16, 1], F32, name=f"rowsum_{br}")
        nc.scalar.activation(d["P"][:, :], d["S_ps"][:, :], AF.Exp,
                             scale=1.0, accum_out=d["rowsum"][:, :])
        d["rinv"] = work.tile([16, 1], F32, name=f"rinv_{br}")
        nc.vector.reciprocal(d["rinv"][:, :], d["rowsum"][:, :])

    def ncol_mm(br):
        d = s1[br]
        d["ncol_ps"] = ps1(br, [TDN, 1], "ncol")
        nc.tensor.matmul(d["ncol_ps"][:, :], d["P"][:, :], d["rinv"][:, :])

    def ncol_cp(br):
        d = s1[br]
        d["ncol"] = work.tile([TDN, 1], BF16, name=f"ncol_{br}")
        nc.vector.tensor_copy(d["ncol"][:, :], d["ncol_ps"][:, :])

    def m2_mm(br):
        d = s1[br]
        d["M2_ps"] = ps1(br, [TDN, 16], "M2")
        nc.tensor.matmul(d["M2_ps"][:, :], d["vphT"][:, :], d["GT"][:, :])

    def m2_cp(br):
        d = s1[br]
        d["M2"] = work.tile([TDN, 16], BF16, name=f"M2_{br}")
        nc.scalar.copy(d["M2"][:, :], d["M2_ps"][:, :])

    def zw_mm(br):
        d = s1[br]
        d["Zw_ps"] = ps1(br, [TDN, OFC], "Zw")
        nc.tensor.matmul(d["Zw_ps"][:, :], d["vphT"][:, :], woT[br])

    def zw_cp(br):
        d = s1[br]
        d["Zw"] = work.tile([TDN, OFC], BF16, name=f"Zw_{br}")
        nc.scalar.copy(d["Zw"][:, :], d["Zw_ps"][:, :])

    def s_mm2(br):
        d = s1[br]
        off = 0 if br == "A" else 16
        nc.tensor.matmul(s_ps[0:1, off:off + 16], d["ncol"][:, :], d["M2"][:, :],
                         start=False, stop=True)

    def sel_post(br):
        d = s1[br]
        off = 0 if br == "A" else 16
        d["m"] = work.tile([1, 1], F32, name=f"m_{br}")
        nc.vector.reduce_max(d["m"][:, :], s_ps[0:1, off:off + 16], axis=X)
        d["oh"] = work.tile([1, 16], F32, name=f"oh_{br}")
        nc.vector.tensor_scalar(d["oh"][:, :], s_ps[0:1, off:off + 16],
                                d["m"][:, :], None, op0=ALU.is_equal)

    def oht_mm(br):
        d = s1[br]
        d["ohT_ps"] = ps1(br, [16, 1], "ohT")
        nc.tensor.transpose(d["ohT_ps"][:, :], d["oh"][:, :], id1)

    def rh_cp(br):
        d = s1[br]
        d["rh"] = work.tile([16, 1], F32, name=f"rh_{br}")
        nc.vector.tensor_mul(d["rh"][:, :], d["ohT_ps"][:, :], d["rinv"][:, :])

    def nsel_mm(br):
        d = s1[br]
        d["nsel_ps"] = ps1(br, [TDN, 1], "nsel")
        nc.tensor.matmul(d["nsel_ps"][:, :], d["P"][:, :], d["rh"][:, :])

    def nsel_cp(br):
        d = s1[br]
        d["nsel"] = work.tile([TDN, 1], BF16, name=f"nsel_{br}")
        nc.vector.tensor_copy(d["nsel"][:, :], d["nsel_ps"][:, :])

    def row_mm(br):
        d = s1[br]
        d["row_ps"] = ps1(br, [1, OFC], "row")
        nc.tensor.matmul(d["row_ps"][:, :], d["nsel"][:, :], d["Zw"][:, :])

    def row_cp(br):
        d = s1[br]
        d["row"] = work.tile([1, OFC], BF16, name=f"row_{br}")
        nc.vector.tensor_add(d["row"][:, :], d["row_ps"][:, :], obrr[br])

    e = {}

    def eproj(nm, col):
        ps = pst([16, OFC], f"{nm}_ps", "tE")
        nc.tensor.matmul(ps[:, :], wb[0:16, col:col + 16], eeg)
        e[nm + "_ps"] = ps

    def eproj_cp(nm):
        t = work.tile([16, OFC], BF16, name=nm)
        nc.scalar.copy(t[:, :], e[nm + "_ps"][:, :])
        e[nm] = t

    def factor_mm(nm, lhsT, rhs, shape):
        ps = pst(shape, f"{nm}_ps", "tE")
        nc.tensor.matmul(ps[:, :], lhsT, rhs)
        e[nm + "_ps"] = ps

    # ---------------- emission order ----------------------------------------
    qk_mms("A")
    qk_mms("B")
    qk_cps("A")
    nc.vector.memset(vpads[:, :], 0.0)
    vphT_mm("A")
    vphT_mm("B")
    qk_cps("B")
    s_mm("A")
    csel_mm("A")
    vphT_cp("A")
    s_mm("B")
    csel_mm("B")
    vphT_cp("B")
    softmax1("A")
    gt_mm("A")
    gt_mm("B")
    softmax1("B")
    gt_cp("A")
    gt_cp("B")
    ncol_mm("A")
    m2_mm("A")
    ncol_cp("A")
    ncol_mm("B")
    m2_mm("B")
    ncol_cp("B")
    m2_cp("A")
    m2_cp("B")
    eproj("qp1T", C_WQ1T)
    zw_mm("A")
    s_mm2("A")
    eproj_cp("qp1T")
    zw_mm("B")
    sel_post("A")
    zw_cp("A")
    eproj("kp0T", C_WK0T)
    s_mm2("B")
    oht_mm("A")
    sel_post("B")
    rh_cp("A")
    zw_cp("B")
    eproj_cp("kp0T")
    nsel_mm("A")
    oht_mm("B")
    nsel_cp("A")
    rh_cp("B")
    eproj("qp2T", C_WQ2T)
    nsel_mm("B")
    eproj_cp("qp2T")
    vp03_ps = pst([OFC, 32], "vp03_ps", "tE")
    nc.tensor.matmul(vp03_ps[:, :], eeg, wb[0:16, C_WV03:C_WV03 + 32])
    nc.vector.tensor_copy(vpads[:, 0:16], vp03_ps[:, 0:16])
    nc.vector.tensor_copy(vpads[:, 240:256], vp03_ps[:, 16:32])
    row_mm("A")
    nsel_cp("B")
    row_cp("A")
    eproj("kp3T", C_WK3T)
    eproj_cp("kp3T")
    factor_mm("g0", wb[0:16, C_UQ0:C_UQ0 + 1], e["kp0T"][:, :], [1, OFC])
    g0 = work.tile([1, OFC], BF16, name="g0")
    nc.vector.tensor_copy(g0[:, :], e["g0_ps"][:, :])
    factor_mm("c0", e["kp0T"][:, :], wb[0:16, C_BQ0:C_BQ0 + 1], [OFC, 1])
    c0 = work.tile([OFC, 1], F32, name="c0")
    nc.scalar.copy(c0[:, :], e["c0_ps"][:, :])
    factor_mm("h1", wb[0:16, C_UK1:C_UK1 + 1], e["qp1T"][:, :], [1, OFC])
    h1 = work.tile([1, OFC], BF16, name="h1")
    nc.vector.tensor_scalar_add(h1[:, :], e["h1_ps"][:, :], kapc[1])
    row_mm("B")
    factor_mm("g3", wb[0:16, C_UQ3:C_UQ3 + 1], e["kp3T"][:, :], [1, OFC])
    g3 = work.tile([1, OFC], BF16, name="g3")
    nc.vector.tensor_copy(g3[:, :], e["g3_ps"][:, :])
    factor_mm("c3", e["kp3T"][:, :], wb[0:16, C_BQ3:C_BQ3 + 1], [OFC, 1])
    c3 = work.tile([OFC, 1], F32, name="c3")
    nc.scalar.copy(c3[:, :], e["c3_ps"][:, :])
    factor_mm("h2", wb[0:16, C_UK2:C_UK2 + 1], e["qp2T"][:, :], [1, OFC])
    h2 = work.tile([1, OFC], BF16, name="h2")
    nc.vector.tensor_scalar_add(h2[:, :], e["h2_ps"][:, :], kapc[2])
    row_cp("B")
    rowS = {"A": s1["A"]["row"], "B": s1["B"]["row"]}

    # rowA/rowB dependent
    vp1_ps = pst([OFC, 16], "vp1_ps", "tS", bufs=1)
    nc.tensor.matmul(vp1_ps[:, :], rowS["A"][:, :], wb[0:1, C_VV1:C_VV1 + 16])
    nc.vector.tensor_copy(vpads[:, 80:96], vp1_ps[:, :])
    pt_ps = [None] * 4
    pt_ps[0] = pst([OFC, OFC], "pt0_ps", "tA")
    nc.tensor.matmul(pt_ps[0][:, :], g0[:, :], rowS["A"][:, :])
    pt_ps[1] = pst([OFC, OFC], "pt1_ps", "tA")
    nc.tensor.matmul(pt_ps[1][:, :], rowS["A"][:, :], h1[:, :])
    vp2_ps = pst([OFC, 16], "vp2_ps", "tS", bufs=1)
    nc.tensor.matmul(vp2_ps[:, :], rowS["B"][:, :], wb[0:1, C_VV2:C_VV2 + 16])
    nc.vector.tensor_copy(vpads[:, 160:176], vp2_ps[:, :])
    pt_ps[2] = pst([OFC, OFC], "pt2_ps", "tB")
    nc.tensor.matmul(pt_ps[2][:, :], rowS["B"][:, :], h2[:, :])
    pt_ps[3] = pst([OFC, OFC], "pt3_ps", "tB")
    nc.tensor.matmul(pt_ps[3][:, :], g3[:, :], rowS["B"][:, :])

    ptall = work.tile([OFC, 4 * OFC], BF16, name="ptall")
    biases = [c0, None, None, c3]
    for i in range(4):
        b = biases[i]
        nc.scalar.activation(ptall[:, OFC * i:OFC * (i + 1)], pt_ps[i][:, :],
                             AF.Exp, bias=(b[:, :] if b is not None else 0.0),
                             scale=1.0)

    ztall_ps = pst([64, OFC], "ztall_ps", "tZ", bufs=1)
    rs_ps = pst([OFC, 4], "rs_ps", "tS", bufs=1)
    ones118 = wb[0:OFC, C_ONES118:C_ONES118 + 1]
    for i in range(4):
        nc.tensor.matmul(rs_ps[:, i:i + 1],
                         ptall[:, OFC * i:OFC * (i + 1)], ones118)
        nc.tensor.matmul(ztall_ps[:, :], vpads[:, 64 * i:64 * (i + 1)],
                         ptall[:, OFC * i:OFC * (i + 1)],
                         start=(i == 0), stop=(i == 3))
    rinv_col = work.tile([OFC, 4], F32, name="rinv_col")
    nc.vector.reciprocal(rinv_col[:, :], rs_ps[:, :])
    r4t_ps = pst([4, OFC], "r4t_ps", "tE")
    nc.tensor.transpose(r4t_ps[:, :], rinv_col[:, :], id118[:, :])
    r4 = work.tile([4, OFC], BF16, name="r4")
    nc.vector.tensor_copy(r4[:, :], r4t_ps[:, :])
    m_ps = pst([64, OFC], "m_ps", "tE")
    nc.tensor.matmul(m_ps[:, :], wb[0:4, C_EE:C_EE + 64], r4[:, :])
    m_sb = work.tile([64, OFC], F32, name="m_sb")
    nc.vector.tensor_copy(m_sb[:, :], m_ps[:, :])
    ztn = work.tile([64, OFC], BF16, name="ztn")
    nc.vector.tensor_mul(ztn[:, :], ztall_ps[:, :], m_sb[:, :])

    # ---------------- conv + head ------------------------------------------
    y_ps = pst([4 * C_OUT, NCONV], "y_ps", "tZ", bufs=1)
    for k in range(KS):
        nc.tensor.matmul(y_ps[:, :],
                         wb[0:64, C_CONV + 40 * k:C_CONV + 40 * (k + 1)],
                         ztn[:, k:k + NCONV], start=(k == 0), stop=(k == KS - 1))
    relu = work.tile([4 * C_OUT, NCONV], F32, name="relu")
    nc.scalar.activation(relu[:, :], y_ps[:, :], AF.Relu,
                         bias=pf[0:40, PF_CCONST:PF_CCONST + 1], scale=1.0)
    feat = work.tile([4 * C_OUT, 1], BF16, name="feat")
    nc.vector.reduce_max(feat[:, :], relu[:, :], axis=X)

    h_ps = pst([40, 1], "h_ps", "tZ", bufs=1)
    nc.tensor.matmul(h_ps[:, :], wb[0:40, C_FC1:C_FC1 + 40], feat[:, :])
    eh = work.tile([40, 1], F32, name="eh")
    nc.scalar.activation(eh[:, :], h_ps[:, :], AF.Exp,
                         bias=pf[0:40, PF_NEGB1:PF_NEGB1 + 1], scale=-1.0)
    eh1 = work.tile([40, 1], F32, name="eh1")
    nc.vector.tensor_scalar(eh1[:, :], eh[:, :], 1.0, None, op0=ALU.add)
    hsb = work.tile([40, 1], BF16, name="hsb")
    with nc.allow_low_precision(reason="bf16 operand for the 2x40 head matmul"):
        nc.vector.reciprocal(hsb[:, :], eh1[:, :])
    o_ps = pst([2, 1], "o_ps", "tZ", bufs=1)
    nc.tensor.matmul(o_ps[:, :], wb[0:40, C_FC2:C_FC2 + 2], hsb[:, :])
    eo = work.tile([2, 1], F32, name="eo")
    nc.scalar.activation(eo[:, :], o_ps[:, :], AF.Exp,
                         bias=pf[0:2, PF_NEGB2:PF_NEGB2 + 1], scale=-1.0)
    eo1 = work.tile([2, 1], F32, name="eo1")
    nc.vector.tensor_scalar(eo1[:, :], eo[:, :], 1.0, None, op0=ALU.add)
    res = work.tile([2, 1], F32, name="res")
    nc.vector.reciprocal(res[:, :], eo1[:, :])
    nc.sync.dma_start(out=out_ap, in_=res[:, :])
    ctx.close()


_CACHE = {}


def build():
    if "nc" in _CACHE:
        return _CACHE["nc"]
    nc = bacc.Bacc("TRN2", target_bir_lowering=False, debug=False,
                   num_devices=N_CORES, num_swdge_queues=4)
    H = {
        "wbx": nc.dram_tensor("wbx", [119, 1241], BF16, kind="ExternalInput"),
        "wlo": nc.dram_tensor("wlo", [64, 719], BF16, kind="ExternalInput"),
        "pfx": nc.dram_tensor("pfx", [48, NPF], F32, kind="ExternalInput"),
    }
    out_t = nc.dram_tensor("out", [1, 2], F32, kind="ExternalOutput")
    with tile.TileContext(nc) as tc:
        _emit(nc, tc, H, out_t.ap())
    nc.compile()
    _CACHE["nc"] = nc
    return nc


def pack_inputs(inputs):
    wbx, wlo, pfx = host_pack(inputs)
    return {"wbx": wbx, "wlo": wlo, "pfx": pfx}


def kernel(**inputs):
    in_map = pack_inputs(inputs)
    nc = build()
    res = run_bass_kernel_spmd(nc, [in_map] * N_CORES,
                               core_ids=list(range(N_CORES)))
    return res.results[0]["out"]
